# revision 1
# baseline (speedup 1.0000x reference)
"""Trainium2 Bass kernel for nn_LrUpsampling (TransformerConv + GraphNorm + cosine gram).

Sharding: 8 cores as (head h = c//2) x (node-half = c%2).
- Attention: each core handles 1 of 4 heads for 2048 of 4096 query nodes,
  with K/V for its head computed over all 4096 source nodes locally.
- GraphNorm moments: pair AllReduce (the two cores of a head hold the two
  node-halves of that head's 512 channels).
- Gram: quad AllGather of Y^T within each node-half group, local partial
  gram for both row-quarters of the head block, pair ReduceScatter to sum
  halves, cosine normalization + relu.

All matmuls run as float32r (PE full-rate fp32-reduced; ~1.5e-4 rel err).
Set MM_FP32R=False to fall back to full fp32 matmuls (4x slower on PE).
"""
import numpy as np

LR, HR, HEADS = 512, 2048, 4
C = HR // HEADS          # 512 per-head channels
N = 2 * HR               # 4096 nodes
NH = N // 2              # 2048 nodes per half
EPS = 1e-5
N_CORES = 8
SCALE = 1.0 / np.sqrt(np.float32(C))

MM_FP32R = True

_RUNNER = None


def _build(mm_fp32r=MM_FP32R, stop_after=None):
    import os
    stop_after = stop_after or os.environ.get("K_STOP_AFTER") or None
    from concourse import bacc, tile, mybir
    from concourse.masks import make_identity

    f32 = mybir.dt.float32
    f32r = mybir.dt.float32r if mm_fp32r else mybir.dt.float32
    AF = mybir.ActivationFunctionType
    ALU = mybir.AluOpType

    PAIRS = [[0, 1], [2, 3], [4, 5], [6, 7]]
    QUADS = [[0, 2, 4, 6], [1, 3, 5, 7]]
    ALL = [list(range(N_CORES))]

    nc = bacc.Bacc("TRN2", target_bir_lowering=False, debug=False,
                   num_devices=N_CORES)

    # ---- I/O ----
    xr = nc.dram_tensor("xr", [LR, N], f32r, kind="ExternalInput")      # full lr_x
    xo = nc.dram_tensor("xo", [LR, NH], f32r, kind="ExternalInput")     # own-half lr_x
    wq = nc.dram_tensor("wq", [LR, C], f32r, kind="ExternalInput")      # head block
    wk = nc.dram_tensor("wk", [LR, C], f32r, kind="ExternalInput")
    wv = nc.dram_tensor("wv", [LR, C], f32r, kind="ExternalInput")
    ws = nc.dram_tensor("ws", [LR, C], f32r, kind="ExternalInput")      # Wskip block
    # per-channel columns for this head block, laid out [p, kind, cc]:
    # 0=bq 1=bk 2=bv+bskip 3=gn_weight 4=gn_bias 5=gn_mean_scale
    cols = nc.dram_tensor("cols", [128, 6, 4], f32, kind="ExternalInput")
    g_out = nc.dram_tensor("g", [256, HR], f32, kind="ExternalOutput")

    with tile.TileContext(nc) as tc:
        import contextlib
        ctx = contextlib.ExitStack()
        with ctx:
            consts = ctx.enter_context(tc.tile_pool(name="consts", bufs=1))
            longp = ctx.enter_context(tc.tile_pool(name="long", bufs=1))
            dram = ctx.enter_context(tc.tile_pool(name="dram", bufs=1, space="DRAM"))

            # ---- constants ----
            ident = consts.tile([128, 128], f32)
            make_identity(nc, ident[:])
            ones_f = consts.tile([128, 1], f32)
            nc.vector.memset(ones_f[:], 1.0)
            ones_col = consts.tile([128, 1], f32r)
            nc.scalar.copy(ones_col[:], ones_f[:])
            onesr_f = consts.tile([1, 128], f32)
            nc.vector.memset(onesr_f[:], 1.0)
            ones_row = consts.tile([1, 128], f32r)
            nc.scalar.copy(ones_row[:], onesr_f[:])
            cols_sb = consts.tile([128, 6, 4], f32)
            nc.sync.dma_start(cols_sb[:], cols.ap())

            v_dram = dram.tile([N, C], f32r)

            hp = ctx.enter_context(tc.tile_pool(name="hp", bufs=1))
            h_sb = hp.tile([128, 4, NH], f32)          # 4MB

            pa_cm = tc.tile_pool(name="pa", bufs=1)
            pa = pa_cm.__enter__()
            kT_sb = pa.tile([128, 4, N], f32r)      # [c-part, cc, m] 8MB
            qT_sb = pa.tile([128, 4, NH], f32r)     # 4MB
            skip_sb = pa.tile([128, 4, NH], f32)    # 4MB

            # ================= Phase 1: projections =================
            with tc.tile_pool(name="p1s", bufs=3) as p1s, \
                 tc.tile_pool(name="p1w", bufs=1) as p1w, \
                 tc.tile_pool(name="p1p", bufs=4, space="PSUM") as p1p:
                wk_sb = p1w.tile([128, 4, C], f32r, tag="w1")
                wv_sb = p1w.tile([128, 4, C], f32r, tag="w2")
                nc.sync.dma_start(wk_sb[:], wk.ap().rearrange("(l p) c -> p l c", p=128))
                nc.sync.dma_start(wv_sb[:], wv.ap().rearrange("(l p) c -> p l c", p=128))

                # kT and v fused: one x-chunk load feeds both projections
                for mm8 in range(8):
                    x_t = p1s.tile([128, 4, 512], f32r, tag="xs")
                    nc.sync.dma_start(
                        x_t[:], xr.ap().rearrange("(l p) m -> p l m", p=128)
                        [:, :, mm8 * 512:(mm8 + 1) * 512])
                    for cc in range(4):
                        ps = p1p.tile([128, 512], f32, tag="ps")
                        for lc in range(4):
                            nc.tensor.matmul(
                                ps[:], wk_sb[:, lc, cc * 128:(cc + 1) * 128],
                                x_t[:, lc, :], start=(lc == 0), stop=(lc == 3))
                        nc.vector.tensor_scalar_add(
                            kT_sb[:, cc, mm8 * 512:(mm8 + 1) * 512], ps[:],
                            cols_sb[:, 1, cc:cc + 1])
                    for sub in range(4):
                        ps = p1p.tile([128, 512], f32, tag="ps")
                        for lc in range(4):
                            nc.tensor.matmul(
                                ps[:], x_t[:, lc, sub * 128:(sub + 1) * 128],
                                wv_sb[:, lc, :], start=(lc == 0), stop=(lc == 3))
                        v_st = p1s.tile([128, 512], f32r, tag="vst")
                        nc.vector.tensor_copy(v_st[:], ps[:])
                        nc.sync.dma_start(
                            v_dram[mm8 * 512 + sub * 128: mm8 * 512 + (sub + 1) * 128, :],
                            v_st[:])
                # qT & skip over own half (reuse weight slots)
                wq_sb = p1w.tile([128, 4, C], f32r, tag="w1", name="wq_sb")
                ws_sb = p1w.tile([128, 4, C], f32r, tag="w2", name="ws_sb")
                nc.sync.dma_start(wq_sb[:], wq.ap().rearrange("(l p) c -> p l c", p=128))
                nc.sync.dma_start(ws_sb[:], ws.ap().rearrange("(l p) c -> p l c", p=128))
                for nn4 in range(4):
                    x_t = p1s.tile([128, 4, 512], f32r, tag="xs")
                    nc.sync.dma_start(
                        x_t[:], xo.ap().rearrange("(l p) m -> p l m", p=128)
                        [:, :, nn4 * 512:(nn4 + 1) * 512])
                    for cc in range(4):
                        ps = p1p.tile([128, 512], f32, tag="ps")
                        for lc in range(4):
                            nc.tensor.matmul(
                                ps[:], wq_sb[:, lc, cc * 128:(cc + 1) * 128],
                                x_t[:, lc, :], start=(lc == 0), stop=(lc == 3))
                        nc.vector.tensor_scalar_add(
                            qT_sb[:, cc, nn4 * 512:(nn4 + 1) * 512], ps[:],
                            cols_sb[:, 0, cc:cc + 1])
                        ps2 = p1p.tile([128, 512], f32, tag="ps")
                        for lc in range(4):
                            nc.tensor.matmul(
                                ps2[:], ws_sb[:, lc, cc * 128:(cc + 1) * 128],
                                x_t[:, lc, :], start=(lc == 0), stop=(lc == 3))
                        nc.vector.tensor_scalar_add(
                            skip_sb[:, cc, nn4 * 512:(nn4 + 1) * 512], ps2[:],
                            cols_sb[:, 2, cc:cc + 1])

            # ================= Phase 2: attention =================
            with tc.tile_pool(name="p2s", bufs=2) as p2s, \
                 tc.tile_pool(name="p2b", bufs=1) as p2b, \
                 tc.tile_pool(name="p2ps", bufs=2, space="PSUM") as p2ps, \
                 tc.tile_pool(name="p2po", bufs=1, space="PSUM") as p2po:
                for nb in range(4):
                    o_ps = [p2po.tile([128, 512], f32, tag=f"o{cc}", name=f"o{nb}_{cc}") for cc in range(4)]
                    den_ps = p2po.tile([1, 512], f32, tag="den")
                    for mb in range(32):
                        if mb % 8 == 0:
                            v8 = p2s.tile([128, 8, 512], f32r, tag="vt",
                                          name=f"v8_{nb}_{mb}")
                            nc.sync.dma_start(
                                v8[:], v_dram[mb * 128:(mb + 8) * 128, :]
                                .rearrange("(i p) c -> p i c", p=128))
                        s_ps = p2ps.tile([128, 512], f32, tag="s")
                        for cc in range(4):
                            nc.tensor.matmul(
                                s_ps[:], kT_sb[:, cc, mb * 128:(mb + 1) * 128],
                                qT_sb[:, cc, nb * 512:(nb + 1) * 512],
                                start=(cc == 0), stop=(cc == 3))
                        e_t = p2s.tile([128, 512], f32r, tag="e")
                        nc.scalar.activation(e_t[:], s_ps[:], AF.Exp,
                                             scale=float(SCALE))
                        for cc in range(4):
                            nc.tensor.matmul(
                                o_ps[cc][:],
                                v8[:, mb % 8, cc * 128:(cc + 1) * 128], e_t[:],
                                start=(mb == 0), stop=(mb == 31))
                        nc.tensor.matmul(den_ps[:], ones_col[:], e_t[:],
                                         start=(mb == 0), stop=(mb == 31))
                    # normalize nb block
                    rec_f = p2b.tile([1, 512], f32, tag="rec")
                    nc.vector.reciprocal(rec_f[:], den_ps[:])
                    rec_r = p2b.tile([1, 512], f32r, tag="recr")
                    nc.scalar.copy(rec_r[:], rec_f[:])
                    bc_ps = p2po.tile([128, 512], f32, tag="bc")
                    nc.tensor.matmul(bc_ps[:], ones_row[:], rec_r[:],
                                     start=True, stop=True)
                    bc_sb = p2b.tile([128, 512], f32, tag="bcs")
                    nc.vector.tensor_copy(bc_sb[:], bc_ps[:])
                    for cc in range(4):
                        nc.vector.tensor_tensor(
                            h_sb[:, cc, nb * 512:(nb + 1) * 512],
                            o_ps[cc][:], bc_sb[:], op=ALU.mult)
                        nc.vector.tensor_tensor(
                            h_sb[:, cc, nb * 512:(nb + 1) * 512],
                            h_sb[:, cc, nb * 512:(nb + 1) * 512],
                            skip_sb[:, cc, nb * 512:(nb + 1) * 512], op=ALU.add)

            pa_cm.__exit__(None, None, None)

            if stop_after == "attn":
                nc.sync.dma_start(
                    g_out.ap().rearrange("(r p) k -> p r k", p=128),
                    h_sb[:, 0:2, :])

            # ================= Phase 3: GraphNorm =================
            if stop_after != "attn":
              with tc.tile_pool(name="p3s", bufs=1) as p3s, \
                 tc.tile_pool(name="p3d", bufs=1, space="DRAM") as p3d:
                scr = p3s.tile([128, NH], f32)           # scratch for squares
                s_col = p3s.tile([128, 2, 4], f32)       # [*, 0, cc]=sum, [*, 1, cc]=sumsq
                for cc in range(4):
                    nc.vector.reduce_sum(
                        s_col[:, 0, cc:cc + 1], h_sb[:, cc, :],
                        axis=mybir.AxisListType.X)
                    nc.scalar.square(scr[:], h_sb[:, cc, :])
                    nc.vector.reduce_sum(s_col[:, 1, cc:cc + 1], scr[:],
                                         axis=mybir.AxisListType.X)
                mom_in = p3d.tile([128, 2, 4], f32)
                mom_out = p3d.tile([128, 2, 4], f32)
                nc.sync.dma_start(mom_in[:], s_col[:])
                if stop_after == "momnc":
                    nc.sync.dma_start(mom_out[:], mom_in[:])
                else:
                    nc.gpsimd.collective_compute(
                        "AllReduce", ALU.add, replica_groups=PAIRS,
                        ins=[mom_in.opt()], outs=[mom_out.opt()])
                nc.sync.dma_start(s_col[:], mom_out[:])

                if stop_after in ("mom", "momnc"):
                    dummy = p3s.tile([128, 2, HR], f32, name="dummy")
                    nc.vector.memset(dummy[:], 0.0)
                    nc.vector.tensor_copy(dummy[:, 0, 0:4], s_col[:, 0, :])
                    nc.sync.dma_start(
                        g_out.ap().rearrange("(r p) k -> p r k", p=128), dummy[:])

                # per-channel affine for Y
                mean = p3s.tile([128, 4], f32)
                nc.scalar.mul(mean[:], s_col[:, 0, :], 1.0 / N)
                ex2 = p3s.tile([128, 4], f32)
                nc.scalar.mul(ex2[:], s_col[:, 1, :], 1.0 / N)
                t_c = p3s.tile([128, 4], f32)
                nc.vector.tensor_tensor(t_c[:], cols_sb[:, 5, :], mean[:], op=ALU.mult)
                u_c = p3s.tile([128, 4], f32)   # 2*mean - t
                nc.scalar.mul(u_c[:], mean[:], 2.0)
                nc.vector.tensor_tensor(u_c[:], u_c[:], t_c[:], op=ALU.subtract)
                var = p3s.tile([128, 4], f32)   # ex2 - t*u
                nc.vector.tensor_tensor(var[:], t_c[:], u_c[:], op=ALU.mult)
                nc.vector.tensor_tensor(var[:], ex2[:], var[:], op=ALU.subtract)
                eps_c = p3s.tile([128, 1], f32)
                nc.vector.memset(eps_c[:], EPS)
                std = p3s.tile([128, 4], f32)
                nc.scalar.activation(std[:], var[:], AF.Sqrt, bias=eps_c[:])
                rstd = p3s.tile([128, 4], f32)
                nc.vector.reciprocal(rstd[:], std[:])
                scaleY = p3s.tile([128, 4], f32)
                nc.vector.tensor_tensor(scaleY[:], rstd[:], cols_sb[:, 3, :], op=ALU.mult)
                biasY = p3s.tile([128, 4], f32)
                nc.vector.tensor_tensor(biasY[:], t_c[:], scaleY[:], op=ALU.mult)
                nc.vector.tensor_tensor(biasY[:], cols_sb[:, 4, :], biasY[:], op=ALU.subtract)

                y_sb = p3s.tile([128, 4, NH], f32)
                for cc in range(4):
                    nc.vector.tensor_scalar(
                        out=y_sb[:, cc, :], in0=h_sb[:, cc, :],
                        scalar1=scaleY[:, cc:cc + 1], scalar2=biasY[:, cc:cc + 1],
                        op0=ALU.mult, op1=ALU.add)

                # ---- row norms (diag of gram): sumsq of Y over own half ----
                diag_c = p3s.tile([128, 4], f32)
                for cc in range(4):
                    nc.scalar.square(scr[:], y_sb[:, cc, :])
                    nc.vector.reduce_sum(diag_c[:, cc:cc + 1], scr[:],
                                         axis=mybir.AxisListType.X)

                # ---- transpose Y -> YT (f32r), stage for AG ----
                yt_sb = p3s.tile([128, 16, C], f32r)
                yt_dram = p3d.tile([NH, C], f32r)
                ygath_a = p3d.tile([4 * NH // 2, C], f32r)
                ygath_b = p3d.tile([4 * NH // 2, C], f32r)
                with tc.tile_pool(name="tp", bufs=4, space="PSUM") as tpp:
                    for cc in range(4):
                        for nn in range(16):
                            tp = tpp.tile([128, 128], f32, tag="tp")
                            nc.tensor.transpose(
                                tp[:], y_sb[:, cc, nn * 128:(nn + 1) * 128], ident[:])
                            nc.vector.tensor_copy(
                                yt_sb[:, nn, cc * 128:(cc + 1) * 128], tp[:])
                nc.sync.dma_start(
                    yt_dram[0:NH // 2, :].rearrange("(nn p) c -> p nn c", p=128),
                    yt_sb[:, 0:8, :])
                nc.sync.dma_start(
                    yt_dram[NH // 2:NH, :].rearrange("(nn p) c -> p nn c", p=128),
                    yt_sb[:, 8:16, :])
                if stop_after == "yt":
                    yb = p3s.tile([128, 512], f32r, name="yb")
                    nc.sync.dma_start(yb[:], yt_dram[0:128, :])
                    ybf = p3s.tile([128, 512], f32, name="ybf")
                    nc.vector.tensor_copy(ybf[:], yb[:])
                    nc.vector.tensor_copy(y_sb[:, 0, 0:512], ybf[:])
                    nc.sync.dma_start(
                        g_out.ap().rearrange("(r p) k -> p r k", p=128),
                        y_sb[:, 0:2, :])
                if stop_after not in ("yt", "mom", "momnc"):
                    nc.gpsimd.collective_compute(
                        "AllGather", ALU.bypass, replica_groups=QUADS,
                        ins=[yt_dram[0:NH // 2, :].opt()], outs=[ygath_a.opt()])
                    nc.gpsimd.collective_compute(
                        "AllGather", ALU.bypass, replica_groups=QUADS,
                        ins=[yt_dram[NH // 2:NH, :].opt()], outs=[ygath_b.opt()])

                if stop_after in ("mom", "momnc", "yt"):
                    pass
                elif stop_after == "ag1":
                    cons = p3s.tile([128, 512], f32r, name="cons")
                    nc.sync.dma_start(cons[:], ygath_a[0:128, :])
                    consf = p3s.tile([128, 512], f32, name="consf")
                    nc.vector.tensor_copy(consf[:], cons[:])
                    nc.vector.tensor_copy(y_sb[:, 0, 0:512], consf[:])
                    nc.sync.dma_start(
                        g_out.ap().rearrange("(r p) k -> p r k", p=128),
                        y_sb[:, 0:2, :])

                # diag collectives (land while gram computes)
                if stop_after not in ("ag1", "mom", "momnc", "yt"):
                    _phase4(nc, tc, mybir, p3s, p3d, y_sb, yt_sb,
                            (ygath_a, ygath_b), diag_c,
                            cols_sb, ones_row, g_out, f32, f32r, stop_after)

    nc.compile()
    return nc


def _phase4(nc, tc, mybir, p3s, p3d, y_sb, yt_sb, ygaths, diag_c, cols_sb,
            ones_row, g_out, f32, f32r, stop_after):
    ygath_a, ygath_b = ygaths
    from concourse import tile
    AF = mybir.ActivationFunctionType
    ALU = mybir.AluOpType
    PAIRS = [[0, 1], [2, 3], [4, 5], [6, 7]]
    ALL = [list(range(N_CORES))]
    if True:
            if True:
                diag8 = p3s.tile([128, 8], f32)
                nc.vector.memset(diag8[:], 0.0)
                nc.vector.tensor_copy(diag8[:, 0:4], diag_c[:])
                diag_in = p3d.tile([128, 8], f32)
                diag_pair = p3d.tile([128, 8], f32)
                dgath = p3d.tile([8, 128, 8], f32)
                nc.sync.dma_start(diag_in[:], diag8[:])
                nc.gpsimd.collective_compute(
                    "AllReduce", ALU.add, replica_groups=PAIRS,
                    ins=[diag_in.opt()], outs=[diag_pair.opt()])
                nc.gpsimd.collective_compute(
                    "AllGather", ALU.bypass, replica_groups=ALL,
                    ins=[diag_pair.opt()], outs=[dgath.opt()])
                # own-block rsqrt (rows scale)
                own_d = p3s.tile([128, 4], f32)
                nc.sync.dma_start(own_d[:], diag_pair[:, 0:4])
                own_std = p3s.tile([128, 4], f32)
                nc.scalar.activation(own_std[:], own_d[:], AF.Sqrt)
                r_own = p3s.tile([128, 4], f32)
                nc.vector.reciprocal(r_own[:], own_std[:])
                # all-channel rsqrt row (cols scale), from even AG slots
                d_row = p3s.tile([1, HR], f32)
                for s2 in range(4):
                    nc.sync.dma_start(
                        d_row[0:1, s2 * 512:(s2 + 1) * 512].rearrange(
                            "one (cc pp) -> one cc pp", cc=4),
                        dgath[2 * s2:2 * s2 + 1, :, 0:4].rearrange(
                            "one pp cc -> one cc pp"))
                nc.scalar.activation(d_row[:], d_row[:], AF.Sqrt)
                rall_row = p3s.tile([1, HR], f32)
                nc.vector.reciprocal(rall_row[:], d_row[:])
                rall_r = p3s.tile([1, HR], f32r)
                nc.scalar.copy(rall_r[:], rall_row[:])

                # ================= Phase 4: gram =================
                rsin_sb = p3s.tile([128, 4, HR], f32)   # row-scaled partials
                with tc.tile_pool(name="p4s", bufs=3) as p4s, \
                     tc.tile_pool(name="p4p", bufs=1, space="PSUM") as p4p:
                    for chf in range(2):   # column half (2 shards each)
                        gp = [[p4p.tile([128, 512], f32, tag=f"g{rr}{s2}", name=f"g{chf}_{rr}{s2}")
                               for s2 in range(2)] for rr in range(4)]
                        for nn in range(16):
                            rt = []
                            gsrc = ygath_a if nn < 8 else ygath_b
                            nnl = nn % 8
                            for s2 in range(2):
                                r_t = p4s.tile([128, 512], f32r, tag=f"rt{s2}")
                                nc.sync.dma_start(
                                    r_t[:],
                                    gsrc[(2 * chf + s2) * (NH // 2) + nnl * 128:
                                         (2 * chf + s2) * (NH // 2) + (nnl + 1) * 128, :])
                                rt.append(r_t)
                            for rr in range(4):      # row chunk = 2*q + jj
                                for s2 in range(2):
                                    nc.tensor.matmul(
                                        gp[rr][s2][:],
                                        yt_sb[:, nn, rr * 128:(rr + 1) * 128],
                                        rt[s2][:],
                                        start=(nn == 0), stop=(nn == 15))
                        for rr in range(4):
                            for s2 in range(2):
                                nc.vector.tensor_scalar_mul(
                                    rsin_sb[:, rr, (2 * chf + s2) * 512:
                                            (2 * chf + s2 + 1) * 512],
                                    gp[rr][s2][:], r_own[:, rr:rr + 1])

                rsin_d = [p3d.tile([512, HR // 2], f32, name=f"rsin{i}")
                          for i in range(2)]
                rsout_d = [p3d.tile([256, HR // 2], f32, name=f"rsout{i}")
                           for i in range(2)]
                for i in range(2):
                    nc.sync.dma_start(
                        rsin_d[i][:].rearrange("(rc p) k -> p rc k", p=128),
                        rsin_sb[:, :, i * (HR // 2):(i + 1) * (HR // 2)])
                    nc.gpsimd.collective_compute(
                        "ReduceScatter", ALU.add, replica_groups=PAIRS,
                        ins=[rsin_d[i].opt()], outs=[rsout_d[i].opt()])

                # column scale + relu + out
                with tc.tile_pool(name="p5p", bufs=1, space="PSUM") as p5p:
                    cs_ps = p5p.tile([128, HR], f32)
                    for s4 in range(4):
                        nc.tensor.matmul(
                            cs_ps[:, s4 * 512:(s4 + 1) * 512], ones_row[:],
                            rall_r[:, s4 * 512:(s4 + 1) * 512],
                            start=True, stop=True)
                    gfin = p3s.tile([128, 2, HR], f32)
                    for i in range(2):
                        nc.sync.dma_start(
                            gfin[:, :, i * (HR // 2):(i + 1) * (HR // 2)],
                            rsout_d[i][:].rearrange("(r p) k -> p r k", p=128))
                    for r in range(2):
                        nc.vector.tensor_tensor(
                            gfin[:, r, :], gfin[:, r, :], cs_ps[:], op=ALU.mult)
                        nc.scalar.activation(gfin[:, r, :], gfin[:, r, :], AF.Relu)
                    nc.sync.dma_start(
                        g_out.ap().rearrange("(r p) k -> p r k", p=128), gfin[:])


def _get_runner():
    global _RUNNER
    if _RUNNER is None:
        import os, sys
        sys.path.insert(0, "/opt/trn_rl_repo")
        sys.path.insert(0, os.path.dirname(os.path.abspath(__file__)))
        nc = _build()
        Runner = _make_runner_cls()
        _RUNNER = Runner(nc, N_CORES)
    return _RUNNER


def _make_runner_cls():
    """Inline fallback runner (kernel.py must be self-contained)."""
    import jax
    from jax.sharding import Mesh, PartitionSpec
    from jax.experimental.shard_map import shard_map
    from concourse import mybir
    from concourse.bass2jax import (_bass_exec_p, install_neuronx_cc_hook,
                                    partition_id_tensor)

    class Runner:
        def __init__(self, nc, n_cores):
            install_neuronx_cc_hook()
            self.nc = nc
            self.n_cores = n_cores
            pname = nc.partition_id_tensor.name if nc.partition_id_tensor else None
            in_names, out_names, out_avals = [], [], []
            for alloc in nc.m.functions[0].allocations:
                if not isinstance(alloc, mybir.MemoryLocationSet):
                    continue
                name = alloc.memorylocations[0].name
                if alloc.kind == "ExternalInput":
                    if name != pname:
                        in_names.append(name)
                elif alloc.kind == "ExternalOutput":
                    out_names.append(name)
                    out_avals.append(jax.core.ShapedArray(
                        tuple(alloc.tensor_shape), mybir.dt.np(alloc.dtype)))
            self.in_names, self.out_names, self.out_avals = in_names, out_names, out_avals
            all_in = list(in_names) + list(out_names)
            if pname is not None:
                all_in.append(pname)

            def _body(*args):
                operands = list(args)
                if pname is not None:
                    operands.append(partition_id_tensor())
                return tuple(_bass_exec_p.bind(
                    *operands, out_avals=tuple(out_avals),
                    in_names=tuple(all_in), out_names=tuple(out_names),
                    lowering_input_output_aliases=(),
                    sim_require_finite=True, sim_require_nnan=True, nc=nc))

            devices = jax.devices()[:n_cores]
            self.mesh = Mesh(np.asarray(devices), ("core",))
            n_args = len(in_names) + len(out_names)
            self.fn = jax.jit(shard_map(
                _body, mesh=self.mesh,
                in_specs=(PartitionSpec("core"),) * n_args,
                out_specs=(PartitionSpec("core"),) * len(out_names),
                check_rep=False))

        def stage(self, in_maps):
            import jax
            per_core = [[np.asarray(m[n]) for n in self.in_names] for m in in_maps]
            concat = [np.concatenate([per_core[c][i] for c in range(self.n_cores)],
                                     axis=0) for i in range(len(self.in_names))]
            zeros = [np.zeros((self.n_cores * a.shape[0], *a.shape[1:]), a.dtype)
                     for a in self.out_avals]
            return [jax.device_put(x) for x in concat + zeros]

        def run_staged(self, staged):
            import jax
            outs = self.fn(*staged)
            jax.block_until_ready(outs)
            return outs

        def run(self, in_maps):
            outs = self.run_staged(self.stage(in_maps))
            res = []
            for c in range(self.n_cores):
                res.append({n: np.asarray(outs[i]).reshape(
                    self.n_cores, *self.out_avals[i].shape)[c]
                    for i, n in enumerate(self.out_names)})
            return res

    return Runner


def make_in_maps(lr_x, Wq, bq, Wk, bk, Wv, bv, Wskip, bskip,
                 gn_weight, gn_bias, gn_mean_scale):
    lr_x = np.asarray(lr_x, np.float32)
    in_maps = []
    for c in range(N_CORES):
        h, half = c // 2, c % 2
        cs = slice(h * C, (h + 1) * C)
        col = np.zeros((128, 6, 4), np.float32)
        for k, vec in enumerate((
                np.asarray(bq)[cs], np.asarray(bk)[cs],
                np.asarray(bv)[cs] + np.asarray(bskip)[cs],
                np.asarray(gn_weight)[cs], np.asarray(gn_bias)[cs],
                np.asarray(gn_mean_scale)[cs])):
            col[:, k, :] = np.asarray(vec, np.float32).reshape(4, 128).T
        in_maps.append({
            "xr": lr_x,
            "xo": np.ascontiguousarray(lr_x[:, half * NH:(half + 1) * NH]),
            "wq": np.ascontiguousarray(np.asarray(Wq, np.float32)[:, cs]),
            "wk": np.ascontiguousarray(np.asarray(Wk, np.float32)[:, cs]),
            "wv": np.ascontiguousarray(np.asarray(Wv, np.float32)[:, cs]),
            "ws": np.ascontiguousarray(np.asarray(Wskip, np.float32)[:, cs]),
            "cols": col,
        })
    return in_maps


def kernel(**inputs):
    runner = _get_runner()
    in_maps = make_in_maps(**inputs)
    res = runner.run(in_maps)
    return np.concatenate([res[c]["g"] for c in range(N_CORES)], axis=0)

